# revision 18
# baseline (speedup 1.0000x reference)
"""LeNet C3 grouped-conv layer as a Trainium2 Bass/Tile kernel.

Math: y[b,o,h,w] = sum_{c,dy,dx} W[o,c,dy,dx] * x[b,c,h+dy,w+dx] + bias[o]
with W the dense 16x6x5x5 weight built from the C3 connectivity tables
(absent connections are zero).

Mapping (per core, 16 images of the batch):
  - One matmul "window" computes S=8 consecutive output rows for all 16
    output channels of TWO images at once.
  - Contraction dim (partitions): (dr, c) with dr in 0..11 input rows of the
    window, c in 0..5 channels -> K = 72.
  - Stationary lhsT[dr*6+c, o*8+s] = W[o, c, dr-s, dx]  (one per dx tap).
  - Moving rhs = x rows tile [72, 512]: cols 0..255 image A rows, 256..511
    image B rows. 5 matmuls (dx = 0..4) accumulate in PSUM with the rhs AP
    shifted by dx columns; out free dim N = 508 >= 256 keeps float32r at
    1 cycle/column.
  - Output cols 0..251 are image A (w=0..251), 256..507 image B, 252..255
    garbage (seam), never copied out.
  - Bias is added during the PSUM->SBUF copy (DVE tensor_scalar_add with a
    per-partition bias vector).
"""

import sys

sys.path.insert(0, "/opt/trn_rl_repo")

import numpy as np

_CH3 = np.array([[0, 1, 2], [1, 2, 3], [2, 3, 4], [3, 4, 5], [0, 4, 5], [0, 1, 5]])
_CH4 = np.array(
    [
        [0, 1, 2, 3],
        [1, 2, 3, 4],
        [2, 3, 4, 5],
        [0, 3, 4, 5],
        [0, 1, 4, 5],
        [0, 1, 2, 5],
        [0, 1, 3, 4],
        [1, 2, 4, 5],
        [0, 2, 3, 5],
    ]
)
_CH6 = np.array([[0, 1, 2, 3, 4, 5]])

_B_PER_CORE = 16  # 128 batch / 8 cores
_N_CORES = 8
_H = 256
_W = 256
_HO = 252
_WO = 252

_module_cache = {}


def _round_f32r(a):
    """Round fp32 to fp32r (e8m11): RNE into 11 mantissa bits, low 12 zero."""
    b = np.ascontiguousarray(a, dtype=np.float32).view(np.uint32)
    lsb = (b >> np.uint32(12)) & np.uint32(1)
    rnd = np.uint32((1 << 11) - 1) + lsb
    b2 = (b + rnd) & np.uint32(0xFFFFF000)
    return b2.view(np.float32)


def _dense_weights(w3, b3, w4, b4, w6, b6):
    W = np.zeros((16, 6, 5, 5), np.float32)
    bias = np.zeros((16,), np.float32)
    for i in range(6):
        W[i, _CH3[i]] = w3[i]
    bias[0:6] = b3
    for i in range(9):
        W[6 + i, _CH4[i]] = w4[i]
    bias[6:15] = b4
    W[15, _CH6[0]] = w6[0]
    bias[15] = np.asarray(b6).reshape(-1)[0]
    return W, bias


def _host_tensors(w3, b3, w4, b4, w6, b6):
    W, bias = _dense_weights(w3, b3, w4, b4, w6, b6)
    # lhsT[(dr, c), (dx, o, s)] = W[o, c, dr - s, dx] where 0 <= dr-s < 5
    lhsT = np.zeros((12, 6, 5, 16, 8), np.float32)
    for dr in range(12):
        for s in range(8):
            dy = dr - s
            if 0 <= dy < 5:
                # [c, dx, o] <- W[o, c, dy, dx]
                lhsT[dr, :, :, :, s] = W[:, :, dy, :].transpose(1, 2, 0)
    # partial (4-row) window: [48, (dx, o, s<4)] packed with one free dim
    lhsTp = np.ascontiguousarray(lhsT[0:8, :, :, :, 0:4].reshape(48, 320))
    lhsT = np.ascontiguousarray(lhsT.reshape(72, 640))
    biasf = np.repeat(bias, 8).reshape(128, 1).astype(np.float32)  # p = o*8+s
    biasp = np.repeat(bias, 4).reshape(64, 1).astype(np.float32)  # p = o*4+s
    return lhsT, lhsTp, biasf, biasp


def _build_module():
    if "nc" in _module_cache:
        return _module_cache["nc"]

    import concourse.bacc as bacc
    import concourse.mybir as mybir
    from concourse.tile import TileContext

    f32 = mybir.dt.float32
    f32r = mybir.dt.float32r

    nc = bacc.Bacc("TRN2", target_bir_lowering=False, debug=False)
    x = nc.dram_tensor("x", [_B_PER_CORE, 6, _H, _W], f32r, kind="ExternalInput").ap()
    lhsT = nc.dram_tensor("lhsT", [72, 640], f32r, kind="ExternalInput").ap()
    lhsTp = nc.dram_tensor("lhsTp", [48, 320], f32r, kind="ExternalInput").ap()
    biasf = nc.dram_tensor("biasf", [128, 1], f32, kind="ExternalInput").ap()
    biasp = nc.dram_tensor("biasp", [64, 1], f32, kind="ExternalInput").ap()
    y = nc.dram_tensor(
        "y", [_B_PER_CORE, 16, _HO, _WO], f32, kind="ExternalOutput"
    ).ap()

    n_blk = 32  # 31 full 8-row windows + one 4-row window per image

    with TileContext(nc) as tc:
        with (
            tc.tile_pool(name="const", bufs=1) as cpool,
            tc.tile_pool(name="xin", bufs=6) as xpool,
            tc.tile_pool(name="oup", bufs=6) as opool,
            tc.tile_pool(name="psum", bufs=8, space="PSUM") as ppool,
        ):
            wt = cpool.tile([72, 640], f32r)
            nc.sync.dma_start(out=wt, in_=lhsT)
            wtp = cpool.tile([48, 320], f32r)
            nc.sync.dma_start(out=wtp, in_=lhsTp)
            bf = cpool.tile([128, 1], f32)
            nc.sync.dma_start(out=bf, in_=biasf)
            bp = cpool.tile([64, 1], f32)
            nc.sync.dma_start(out=bp, in_=biasp)

            for pair in range(_B_PER_CORE // 2):
                bA = 2 * pair
                for blk in range(n_blk):
                    h0 = 8 * blk
                    S = 8 if blk < n_blk - 1 else 4
                    rows = S + 4
                    P = rows * 6
                    M = 16 * S
                    xt = xpool.tile([72, 512], f32r)
                    for img_i in range(2):
                        # 3D AP [rows, 6, 256]; traversal (h, c, w) maps onto
                        # out partitions p = h*6 + c.
                        src = x[bA + img_i].rearrange("c h w -> h c w")[
                            h0 : h0 + rows
                        ]
                        nc.sync.dma_start(
                            out=xt[0:P, img_i * 256 : (img_i + 1) * 256], in_=src
                        )
                    ps = ppool.tile([128, 508], f32)
                    for dx in range(5):
                        if S == 8:
                            lw = wt[:, dx * 128 : (dx + 1) * 128]
                        else:
                            lw = wtp[:, dx * 64 : (dx + 1) * 64]
                        nc.tensor.matmul(
                            ps[0:M, :],
                            lw,
                            xt[0:P, dx : dx + 508],
                            start=(dx == 0),
                            stop=(dx == 4),
                        )
                    ot = opool.tile([128, 508], f32)
                    bias_ap = bf if S == 8 else bp
                    nc.vector.tensor_scalar_add(ot[0:M, :], ps[0:M, :], bias_ap)
                    for img_i in range(2):
                        # 3D AP [16, S, 252]; traversal (o, s, w) maps from
                        # in_ partitions p = o*S + s.
                        dst = y[bA + img_i][:, h0 : h0 + S, :]
                        nc.sync.dma_start(
                            out=dst, in_=ot[0:M, img_i * 256 : img_i * 256 + 252]
                        )

    nc.compile()
    _module_cache["nc"] = nc
    return nc


def _run(inputs, trace=False):
    from concourse.bass_utils import run_bass_kernel_spmd

    x = _round_f32r(np.asarray(inputs["x"], dtype=np.float32))
    lhsT, lhsTp, biasf, biasp = _host_tensors(
        np.asarray(inputs["w3"], np.float32),
        np.asarray(inputs["b3"], np.float32),
        np.asarray(inputs["w4"], np.float32),
        np.asarray(inputs["b4"], np.float32),
        np.asarray(inputs["w6"], np.float32),
        np.asarray(inputs["b6"], np.float32),
    )
    lhsT = _round_f32r(lhsT)
    lhsTp = _round_f32r(lhsTp)
    nc = _build_module()
    in_maps = [
        {
            "x": np.ascontiguousarray(x[_B_PER_CORE * i : _B_PER_CORE * (i + 1)]),
            "lhsT": lhsT,
            "lhsTp": lhsTp,
            "biasf": biasf,
            "biasp": biasp,
        }
        for i in range(_N_CORES)
    ]
    res = run_bass_kernel_spmd(
        nc, in_maps, core_ids=list(range(_N_CORES)), trace=trace
    )
    out = np.concatenate([res.results[i]["y"] for i in range(_N_CORES)], axis=0)
    return out, res


def kernel(**inputs):
    out, _ = _run(inputs, trace=False)
    return out


# revision 37
# speedup vs baseline: 9.7897x; 9.7897x over previous
"""LeNet C3 grouped-conv layer as a Trainium2 Bass/Tile kernel.

Math: y[b,o,h,w] = sum_{c,dy,dx} W[o,c,dy,dx] * x[b,c,h+dy,w+dx] + bias[o]
with W the dense 16x6x5x5 weight built from the C3 connectivity tables
(absent connections are zero).

Mapping (per core, 16 images of the batch):
  - Input tile: 20 consecutive rows x 6 channels of 4 images, partitions
    p = c*20 + dr (c-major; natural HBM order -> one 2D/3D 120 KB DMA per
    image per block), cols = img*256 + w.  K = 120.
  - Each 20-row block yields 16 output rows via TWO window phases: phase
    ph covers output rows h0+8*ph .. h0+8*ph+7.  Phase selection happens
    in the stationary lhsT: lhsT_ph[(c,dr), (o,s)] = W[o, c, dr-8*ph-s, dx]
    (zero outside the band).  Matmul cost is N-bound, so the K=120 zero
    padding is free.
  - 5 matmuls (dx = 0..4) accumulate in PSUM with the moving AP shifted by
    dx columns; images are processed in pairs: N = 508 (>=256 keeps
    float32r at 1 cycle/column).  Out cols 0..251 img A, 256..507 img B,
    252..255 garbage seam (skipped on store).
  - Bias is added during the PSUM->SBUF copy (DVE tensor_scalar_add).
  - Blocks: h0 = 0,16,...,224, then a final block at h0 = 236 (rows
    236..255) producing rows 236..251; rows 236..239 would duplicate
    block 14's output, so that phase stores only s = 4..7 (rows 240..243).
  - float32r (e8m11) inputs, pre-rounded on the host.
"""

import sys

sys.path.insert(0, "/opt/trn_rl_repo")

import numpy as np

_CH3 = np.array([[0, 1, 2], [1, 2, 3], [2, 3, 4], [3, 4, 5], [0, 4, 5], [0, 1, 5]])
_CH4 = np.array(
    [
        [0, 1, 2, 3],
        [1, 2, 3, 4],
        [2, 3, 4, 5],
        [0, 3, 4, 5],
        [0, 1, 4, 5],
        [0, 1, 2, 5],
        [0, 1, 3, 4],
        [1, 2, 4, 5],
        [0, 2, 3, 5],
    ]
)
_CH6 = np.array([[0, 1, 2, 3, 4, 5]])

_B_PER_CORE = 16  # 128 batch / 8 cores
_N_CORES = 8
_H = 256
_W = 256
_HO = 252
_WO = 252
_R = 20  # input rows per block
_K = 6 * _R  # 120 contraction partitions

_module_cache = {}


def _round_f32r(a):
    """Round fp32 to fp32r (e8m11): RNE into 11 mantissa bits, low 12 zero."""
    b = np.ascontiguousarray(a, dtype=np.float32).view(np.uint32)
    lsb = (b >> np.uint32(12)) & np.uint32(1)
    rnd = np.uint32((1 << 11) - 1) + lsb
    b2 = (b + rnd) & np.uint32(0xFFFFF000)
    return b2.view(np.float32)


def _dense_weights(w3, b3, w4, b4, w6, b6):
    W = np.zeros((16, 6, 5, 5), np.float32)
    bias = np.zeros((16,), np.float32)
    for i in range(6):
        W[i, _CH3[i]] = w3[i]
    bias[0:6] = b3
    for i in range(9):
        W[6 + i, _CH4[i]] = w4[i]
    bias[6:15] = b4
    W[15, _CH6[0]] = w6[0]
    bias[15] = np.asarray(b6).reshape(-1)[0]
    return W, bias


def _host_tensors(w3, b3, w4, b4, w6, b6):
    W, bias = _dense_weights(w3, b3, w4, b4, w6, b6)
    # lhsT[(c, dr), (ph, dx, o, s)] = W[o, c, dr - 8*ph - s, dx]
    lhsT = np.zeros((6, _R, 2, 5, 16, 8), np.float32)
    for dr in range(_R):
        for ph in range(2):
            for s in range(8):
                dy = dr - 8 * ph - s
                if 0 <= dy < 5:
                    # [c, dx, o] <- W[o, c, dy, dx]
                    lhsT[:, dr, ph, :, :, s] = W[:, :, dy, :].transpose(1, 2, 0)
    lhsT = np.ascontiguousarray(lhsT.reshape(_K, 2 * 5 * 128))
    # final-block phase-0 variant: output rows h0+4+s (s = 0..3), compact
    # M = 64 with p = o*4 + s.
    lhsTp = np.zeros((6, _R, 5, 16, 4), np.float32)
    for dr in range(_R):
        for s in range(4):
            dy = dr - 4 - s
            if 0 <= dy < 5:
                lhsTp[:, dr, :, :, s] = W[:, :, dy, :].transpose(1, 2, 0)
    lhsTp = np.ascontiguousarray(lhsTp.reshape(_K, 320))
    biasf = np.repeat(bias, 8).reshape(128, 1).astype(np.float32)  # p = o*8+s
    biasp = np.repeat(bias, 4).reshape(64, 1).astype(np.float32)  # p = o*4+s
    return lhsT, lhsTp, biasf, biasp


def _build_module(reps=1):
    if ("nc", reps) in _module_cache:
        return _module_cache[("nc", reps)]

    import concourse.bacc as bacc
    import concourse.mybir as mybir
    from concourse.tile import TileContext

    f32 = mybir.dt.float32
    f32r = mybir.dt.float32r

    nc = bacc.Bacc("TRN2", target_bir_lowering=False, debug=False)
    x = nc.dram_tensor("x", [_B_PER_CORE, 6, _H, _W], f32r, kind="ExternalInput").ap()
    lhsT = nc.dram_tensor("lhsT", [_K, 1280], f32r, kind="ExternalInput").ap()
    lhsTp = nc.dram_tensor("lhsTp", [_K, 320], f32r, kind="ExternalInput").ap()
    biasf = nc.dram_tensor("biasf", [128, 1], f32, kind="ExternalInput").ap()
    biasp = nc.dram_tensor("biasp", [64, 1], f32, kind="ExternalInput").ap()
    y = nc.dram_tensor(
        "y", [_B_PER_CORE, 16, _HO, _WO], f32, kind="ExternalOutput"
    ).ap()

    n_blk = 16  # 15 blocks at h0=16k + final block at h0=236

    with TileContext(nc) as tc:
        with (
            tc.tile_pool(name="const", bufs=1) as cpool,
            tc.tile_pool(name="xin", bufs=8) as xpool,
            tc.tile_pool(name="oup", bufs=6) as opool,
            tc.tile_pool(name="psum", bufs=8, space="PSUM") as ppool,
        ):
            wt = cpool.tile([_K, 1280], f32r)
            nc.sync.dma_start(out=wt, in_=lhsT)
            wtp = cpool.tile([_K, 320], f32r)
            nc.sync.dma_start(out=wtp, in_=lhsTp)
            bf = cpool.tile([128, 1], f32)
            nc.sync.dma_start(out=bf, in_=biasf)
            bp = cpool.tile([64, 1], f32)
            nc.sync.dma_start(out=bp, in_=biasp)

            out_ctr = 0
            for rep in range(reps):
              for quad in range(_B_PER_CORE // 4):
                bQ = 4 * quad
                for blk in range(n_blk):
                    h0 = 16 * blk if blk < n_blk - 1 else 236
                    xt = xpool.tile([_K, 1024], f32r)
                    for img in range(4):
                        # natural HBM order (c, h, w) pairs with partitions
                        # p = c*20 + dr; (h, w) rows are HBM-contiguous.
                        nc.gpsimd.dma_start(
                            out=xt[:, img * 256 : (img + 1) * 256],
                            in_=x[bQ + img][:, h0 : h0 + _R, :],
                        )
                    # col = ph*1024 + img*256 + w
                    ot = opool.tile([128, 2048], f32)
                    final0 = blk == n_blk - 1
                    for ph in range(2):
                        M = 64 if (final0 and ph == 0) else 128
                        for g in range(2):
                            ps = ppool.tile([128, 508], f32)
                            for dx in range(5):
                                if M == 64:
                                    lw = wtp[:, dx * 64 : (dx + 1) * 64]
                                else:
                                    lw = wt[
                                        :,
                                        (ph * 5 + dx) * 128 : (ph * 5 + dx + 1) * 128,
                                    ]
                                nc.tensor.matmul(
                                    ps[0:M, :],
                                    lw,
                                    xt[:, 512 * g + dx : 512 * g + dx + 508],
                                    start=(dx == 0),
                                    stop=(dx == 4),
                                )
                            nc.vector.tensor_scalar_add(
                                ot[
                                    0:M,
                                    ph * 1024 + 512 * g : ph * 1024 + 512 * g + 508,
                                ],
                                ps[0:M, :],
                                bf if M == 128 else bp,
                            )
                    for img in range(4):
                        b = bQ + img
                        for ph in range(2):
                            col0 = ph * 1024 + img * 256
                            eng = nc.scalar if out_ctr % 2 == 0 else nc.sync
                            out_ctr += 1
                            if blk == n_blk - 1 and ph == 0:
                                # compact M=64 variant: rows 240..243 at
                                # partitions p = o*4 + s.
                                eng.dma_start(
                                    out=y[b][:, 240:244, :],
                                    in_=ot[0:64, col0 : col0 + 252],
                                )
                            else:
                                eng.dma_start(
                                    out=y[b][:, h0 + 8 * ph : h0 + 8 * ph + 8, :],
                                    in_=ot[:, col0 : col0 + 252],
                                )

    nc.compile()
    _module_cache[("nc", reps)] = nc
    return nc


def _run(inputs, trace=False):
    from concourse.bass_utils import run_bass_kernel_spmd

    x = _round_f32r(np.asarray(inputs["x"], dtype=np.float32))
    lhsT, lhsTp, biasf, biasp = _host_tensors(
        np.asarray(inputs["w3"], np.float32),
        np.asarray(inputs["b3"], np.float32),
        np.asarray(inputs["w4"], np.float32),
        np.asarray(inputs["b4"], np.float32),
        np.asarray(inputs["w6"], np.float32),
        np.asarray(inputs["b6"], np.float32),
    )
    lhsT = _round_f32r(lhsT)
    lhsTp = _round_f32r(lhsTp)
    nc = _build_module()
    in_maps = [
        {
            "x": np.ascontiguousarray(x[_B_PER_CORE * i : _B_PER_CORE * (i + 1)]),
            "lhsT": lhsT,
            "lhsTp": lhsTp,
            "biasf": biasf,
            "biasp": biasp,
        }
        for i in range(_N_CORES)
    ]
    res = run_bass_kernel_spmd(
        nc, in_maps, core_ids=list(range(_N_CORES)), trace=trace
    )
    out = np.concatenate([res.results[i]["y"] for i in range(_N_CORES)], axis=0)
    return out, res


def kernel(**inputs):
    out, _ = _run(inputs, trace=False)
    return out
